# revision 28
# baseline (speedup 1.0000x reference)
"""Causal multi-head attention (B=8, T=1024, E=768, H=12, D=64) on 8 trn2
NeuronCores, data-parallel over the batch (one batch element per core).

All matmul operands are bf16 (PSUM accumulation fp32): full 1 cycle/column PE
rate at any tile width, half the SBUF/DMA footprint of fp32r, ~0.5% rel err.

Per-core pipeline:
  1. Q^T = Wq @ x^T + bq, K^T likewise  -> SBUF [768, 1024] bf16.
  2. V = x @ Wv^T (bias folded into the output projection) -> SBUF "V65"
     [1024, 12*65] bf16 with a ones column per head so the attention-context
     matmul also produces the softmax denominator.
  3. Per head pair: S^T[k,q] tiles via row-packed matmuls (K=64 contraction,
     tile_position (0,0)/(64,0)) into PSUM fp32, causal mask added on the
     diagonal 128-block (one DVE op for both heads via a stride-0 AP),
     exp on ACT (scale=1/8) -> bf16, ctx^T[65,q] accumulated with V65
     stationary.  S matmuls run one k-tile AHEAD of ctx so PE doesn't wait
     on the DVE/ACT chain, and projection matmuls for the NEXT head pair are
     interleaved one "unit" per k-step so PE has filler work during exp.
  4. Window-end softmax normalization is DEFERRED: Pool copies ctx PSUM ->
     SBUF (frees PSUM), then 2 k-steps later DVE reciprocals the denominator
     row and the ACT HWDGE ring bounces it through DRAM to broadcast across
     64 partitions, then 3 more k-steps later DVE/Pool multiply it in.  No
     engine queue ever blocks on the DMA round-trip.
  5. out = ctx_norm @ Wo^T + bo_eff (bo_eff = bo + bv @ Wo^T), PSUM
     double-buffered by alternating the pp pool with an stp bank pair.

The sharding/gather and all host-side layout prep (transposes, bf16 casts,
bias precomputation) happen in kernel() below.
"""
import sys
import numpy as np

sys.path.insert(0, "/opt/trn_rl_repo")

import concourse.bass as bass
import concourse.mybir as mybir
import concourse.tile as tile

F32 = mybir.dt.float32
F32R = mybir.dt.float32r
BF16 = mybir.dt.bfloat16
NP_BF16 = mybir.dt.np(BF16)

# matmul datapath dtype: False -> float32r (single fused Matmult instruction,
# no Ldweights), True -> bf16 (half SBUF/DMA footprint, 2x instruction count)
MM_BF16 = False
DT = BF16 if MM_BF16 else F32R
DRAM_DT = BF16 if MM_BF16 else F32
NP_DT = NP_BF16 if MM_BF16 else np.float32


def _mm(ap):
    """view a DRAM f32 AP as f32r for matmul-bound DMA loads"""
    return ap if MM_BF16 else ap.bitcast(F32R)

B, T, E, H, D = 8, 1024, 768, 12, 64
NCH = E // 128          # 6 e-chunks
NTC = T // 128          # 8 t-chunks
NW = T // 512           # 2 q-windows
SCALE = 1.0 / np.sqrt(D)
NEG = -1.0e9


def _split_excess_waits(nc, max_waits: int = 1):
    """walrus on this stack accepts at most one embedded sync-wait per
    instruction; peel extras onto wait-only NoOps on the same engine."""
    for func in nc.m.functions:
        for bb in func.blocks:
            insts = bb.instructions
            i = 0
            while i < len(insts):
                inst = insts[i]
                si = getattr(inst, "sync_info", None)
                if si is None or len(si.on_wait) <= max_waits:
                    i += 1
                    continue
                waits = list(si.on_wait)
                keep, extra = waits[:max_waits], waits[max_waits:]
                nops = []
                while extra:
                    chunk, extra = extra[:max_waits], extra[max_waits:]
                    nop = mybir.InstNoOp(
                        name=f"{inst.name}_ws{len(nops)}", ins=[], outs=[])
                    nop.engine = inst.engine
                    nop.sync_info = mybir.SyncInfo(on_wait=chunk, on_update=[])
                    nc.register_instruction(nop, overwrite=True)
                    nops.append(nop)
                si.on_wait = keep
                for j, nop in enumerate(nops):
                    insts.insert(i + j, nop)
                i += len(nops) + 1


def build_nc(repeat: int = 1):
    nc = bass.Bass()
    xT = nc.dram_tensor("xT", [E, T], DRAM_DT, kind="ExternalInput")
    wq_r = nc.dram_tensor("wq_r", [NCH, 128, NCH, 128], DRAM_DT, kind="ExternalInput")
    wk_r = nc.dram_tensor("wk_r", [NCH, 128, NCH, 128], DRAM_DT, kind="ExternalInput")
    wvT = nc.dram_tensor("wvT", [E, E], DRAM_DT, kind="ExternalInput")
    woT = nc.dram_tensor("woT", [E, E], DRAM_DT, kind="ExternalInput")
    bq_pm = nc.dram_tensor("bq_pm", [128, NCH], F32, kind="ExternalInput")
    bk_pm = nc.dram_tensor("bk_pm", [128, NCH], F32, kind="ExternalInput")
    bo_bc = nc.dram_tensor("bo_bc", [128, E], F32, kind="ExternalInput")
    out = nc.dram_tensor("out", [T, E], F32, kind="ExternalOutput")

    tril = np.where(np.arange(128)[None, :] >= np.arange(128)[:, None],
                    0.0, NEG).astype(np.float32)
    maskc = nc.inline_tensor(tril, name="maskc")
    ones12 = nc.inline_tensor(np.ones((128, H), NP_DT), name="ones12")
    ones64c = nc.inline_tensor(np.ones((1, 64), NP_BF16), name="ones64c")

    with tile.TileContext(nc) as tc, \
            nc.allow_low_precision(reason="bf16 pipeline; fp32 PSUM accumulate"):
        from contextlib import ExitStack
        with ExitStack() as ctx:
            consts = ctx.enter_context(tc.tile_pool(name="consts", bufs=2))
            persist = ctx.enter_context(tc.tile_pool(name="persist", bufs=1))
            xt_p = ctx.enter_context(tc.tile_pool(name="xtp", bufs=2 if MM_BF16 else 1))
            wqk_p = ctx.enter_context(tc.tile_pool(name="wqk", bufs=3))
            wrow_p = ctx.enter_context(tc.tile_pool(name="wrow", bufs=6))
            pt_p = ctx.enter_context(tc.tile_pool(name="pt", bufs=4))
            rt_p = ctx.enter_context(tc.tile_pool(name="rt", bufs=4))
            out_p = ctx.enter_context(tc.tile_pool(name="outp", bufs=3))
            pp = ctx.enter_context(tc.tile_pool(name="pp", bufs=2, space="PSUM"))
            stp = ctx.enter_context(tc.tile_pool(name="stp", bufs=2, space="PSUM"))
            ctxp = ctx.enter_context(tc.tile_pool(name="ctxp", bufs=2, space="PSUM"))
            bc_p = ctx.enter_context(tc.tile_pool(name="bcp", bufs=4))

            def body():
                mask_sb = consts.tile([128, 128], F32, tag="cmask")
                bqs = consts.tile([128, NCH], F32, tag="cbq")
                bks = consts.tile([128, NCH], F32, tag="cbk")
                bos = consts.tile([128, E], F32, tag="cbo")
                ones64 = consts.tile([1, 64], BF16, tag="cone")

                # x load split by window-half x 3-chunk groups across the two
                # HWDGE rings so the first projection matmuls start early
                # (weight DMAs are interleaved ahead of the x halves below).
                xt = xt_p.tile([128, NCH, T], DT, tag="xt")

                def xt_dma(eng, win, chlo):
                    base = xT[chlo * 128:(chlo + 3) * 128,
                              win * 512:(win + 1) * 512]
                    eng.dma_start(
                        out=xt[:, chlo:chlo + 3, win * 512:(win + 1) * 512],
                        in_=_mm(bass.AP(tensor=base.tensor, offset=base.offset,
                                        ap=[[T, 128], [128 * T, 3], [1, 512]])))

                qt_sb = persist.tile([128, NCH, T], DT)
                kt_sb = persist.tile([128, NCH, T], DT)
                v65_sb = persist.tile([128, NTC, H * 65], DT)
                ctxT_sb = persist.tile([128, NCH, T], DT)

                # ---- projection building blocks ----
                def qk_win(w, m, win, bias_sb, dst_sb, copy_eng):
                    ps = pp.tile([128, 512], F32, tag="pp")
                    for ch in range(NCH):
                        nc.tensor.matmul(
                            ps, w[:, ch, :], xt[:, ch, win * 512:(win + 1) * 512],
                            start=(ch == 0), stop=(ch == NCH - 1))
                    # only ACT and DVE can read PSUM: q -> ACT, k -> DVE
                    if copy_eng == "act":
                        nc.scalar.activation(
                            dst_sb[:, m, win * 512:(win + 1) * 512], ps,
                            mybir.ActivationFunctionType.Identity,
                            bias=bias_sb[:, m:m + 1])
                    else:
                        nc.vector.tensor_scalar_add(
                            dst_sb[:, m, win * 512:(win + 1) * 512], ps,
                            bias_sb[:, m:m + 1])

                def make_qk_units(m):
                    """6 closures: [q dma, q win0, q win1, k dma, k win0, k win1]"""
                    hold = {}

                    def dma(which, w_r):
                        w = wqk_p.tile([128, NCH, 128], DT, tag="wqk")
                        nc.sync.dma_start(out=w, in_=_mm(w_r[m, :, :, :]))
                        hold[which] = w

                    units = []
                    for which, w_r, bias_sb, dst_sb, eng in (
                            ("q", wq_r, bqs, qt_sb, "act"),
                            ("k", wk_r, bks, kt_sb, "dve")):
                        units.append(
                            lambda which=which, w_r=w_r: dma(which, w_r))
                        for win in range(NW):
                            units.append(
                                lambda which=which, win=win, bias_sb=bias_sb,
                                dst_sb=dst_sb, eng=eng:
                                qk_win(hold[which], m, win, bias_sb, dst_sb, eng))
                    return units

                def proj_v(kc):
                    if kc % 2 == 0:
                        ps0 = pp.tile([128, 512], F32, tag="pp")
                        ps1 = pp.tile([128, 256], F32, tag="pp")
                    else:
                        stv = stp.tile([128, 1024], F32, tag="st")
                        ps0 = stv[:, 0:512]
                        ps1 = stv[:, 512:768]
                    for ch in range(NCH):
                        lhsT = xt[:, ch, kc * 128:(kc + 1) * 128]
                        nc.tensor.matmul(ps0, lhsT, wv_t[ch][:, 0:512],
                                         start=(ch == 0), stop=(ch == NCH - 1))
                        nc.tensor.matmul(ps1, lhsT, wv_t[ch][:, 512:768],
                                         start=(ch == 0), stop=(ch == NCH - 1))
                    v65_r = v65_sb[:, kc, :].rearrange("p (h e) -> p h e", e=65)
                    nc.scalar.copy(v65_r[:, 0:8, 0:64], ps0)
                    nc.scalar.copy(v65_r[:, 8:12, 0:64], ps1)

                # ---- deferred-op scheduler ----
                defq = []      # list of [delay_in_ksteps, closure]
                units_q = []   # projection units to interleave into attn

                def tick():
                    rest = []
                    for item in defq:
                        item[0] -= 1
                        if item[0] <= 0:
                            item[1]()
                        else:
                            rest.append(item)
                    defq[:] = rest

                def pop_unit():
                    if units_q:
                        units_q.pop(0)()

                def flush_all():
                    while units_q:
                        units_q.pop(0)()
                    # run remaining deferred in order, preserving their lag
                    while defq:
                        tick()

                # ---- attention ----
                def attn_pair(p, wins=tuple(range(NW))):
                    hA, hB = 2 * p, 2 * p + 1
                    for win in wins:
                        nk = 4 * (win + 1)
                        # full-bank tiles: rows 0:65 accumulate ctx + denom,
                        # rows 64:128 receive the PE-broadcast reciprocal.
                        ctxA = ctxp.tile([128, 512], F32, tag="ctx")
                        ctxB = ctxp.tile([128, 512], F32, tag="ctx")

                        def emit_ctx(kc, off, pt):
                            nc.tensor.matmul(
                                ctxA[0:65, off:512],
                                v65_sb[:, kc, hA * 65:hA * 65 + 65],
                                pt[:, off:512],
                                start=(kc == 0), stop=(kc == nk - 1))
                            nc.tensor.matmul(
                                ctxB[0:65, off:512],
                                v65_sb[:, kc, hB * 65:hB * 65 + 65],
                                pt[:, 512 + off:1024],
                                start=(kc == 0), stop=(kc == nk - 1))

                        pendq = []
                        for kc in range(nk):
                            off = max(kc * 128 - win * 512, 0)
                            w0 = win * 512
                            st = stp.tile([128, 1024], F32, tag="st")
                            nc.tensor.matmul(
                                st[:, off:512],
                                kt_sb[0:64, p, kc * 128:(kc + 1) * 128],
                                qt_sb[0:64, p, w0 + off:w0 + 512],
                                start=True, stop=True, tile_position=(0, 0))
                            nc.tensor.matmul(
                                st[:, 512 + off:1024],
                                kt_sb[64:128, p, kc * 128:(kc + 1) * 128],
                                qt_sb[64:128, p, w0 + off:w0 + 512],
                                start=True, stop=True, tile_position=(64, 0))
                            if kc * 128 - win * 512 >= 0:  # diagonal block
                                st3 = st.rearrange("p (s q) -> p s q", s=2)
                                mask_b = bass.AP(
                                    tensor=mask_sb.tensor, offset=mask_sb.offset,
                                    ap=[list(mask_sb.ap)[0], [0, 2],
                                        list(mask_sb.ap)[1]])
                                nc.vector.tensor_tensor(
                                    out=st3[:, :, off:off + 128],
                                    in0=st3[:, :, off:off + 128],
                                    in1=mask_b, op=mybir.AluOpType.add)
                            pt = pt_p.tile([128, 1024], DT, tag="pt")
                            if off > 0:
                                st3 = st.rearrange("p (s q) -> p s q", s=2)
                                pt3 = pt.rearrange("p (s q) -> p s q", s=2)
                                nc.scalar.activation(
                                    pt3[:, :, off:512], st3[:, :, off:512],
                                    mybir.ActivationFunctionType.Exp, scale=SCALE)
                            else:
                                nc.scalar.activation(
                                    pt, st, mybir.ActivationFunctionType.Exp,
                                    scale=SCALE)
                            if len(pendq) >= 2:
                                emit_ctx(*pendq.pop(0))
                            pendq.append((kc, off, pt))
                            tick()
                            pop_unit()
                        while pendq:
                            emit_ctx(*pendq.pop(0))
                            pop_unit()
                        # softmax normalization: DVE reciprocals of the
                        # denominator rows, broadcast across partitions by a
                        # K=1 matmul into the free rows 64:128 of the same ctx
                        # PSUM bank (PE col offset 64), DVE-copied to SBUF,
                        # then deferred PSUMxSBUF multiplies.
                        bcA = bc_p.tile([64, 512], F32, tag="bc")
                        bcB = bc_p.tile([64, 512], F32, tag="bc")

                        def c_recip(ctxA=ctxA, ctxB=ctxB):
                            rtA = rt_p.tile([1, 512], BF16, tag="rt")
                            rtB = rt_p.tile([1, 512], BF16, tag="rt")
                            nc.vector.reciprocal(rtA, ctxA[64:65, :])
                            nc.vector.reciprocal(rtB, ctxB[64:65, :])
                            nc.tensor.matmul(ctxA[64:128, :], ones64[0:1, :],
                                             rtA[0:1, :], start=True, stop=True,
                                             tile_position=(0, 64))
                            nc.tensor.matmul(ctxB[64:128, :], ones64[0:1, :],
                                             rtB[0:1, :], start=True, stop=True,
                                             tile_position=(0, 64))

                        def c_copy(ctxA=ctxA, ctxB=ctxB, bcA=bcA, bcB=bcB):
                            nc.vector.tensor_copy(bcA, ctxA[64:128, :])
                            nc.vector.tensor_copy(bcB, ctxB[64:128, :])

                        def c_mult(p=p, win=win, ctxA=ctxA, ctxB=ctxB,
                                   bcA=bcA, bcB=bcB):
                            nc.vector.tensor_tensor(
                                out=ctxT_sb[0:64, p, win * 512:(win + 1) * 512],
                                in0=ctxA[0:64, :], in1=bcA,
                                op=mybir.AluOpType.mult)
                            nc.vector.tensor_tensor(
                                out=ctxT_sb[64:128, p, win * 512:(win + 1) * 512],
                                in0=ctxB[0:64, :], in1=bcB,
                                op=mybir.AluOpType.mult)

                        defq.append([1, c_recip])
                        defq.append([2, c_copy])
                        defq.append([3, c_mult])
                        tick()

                # ---- emission order ----
                # DMA ring order matters: q weight first on sync, then the x
                # halves PE needs first; remaining x goes on the ACT ring.
                qk0 = make_qk_units(0)
                qk0[0]()                      # wq DMA (sync)
                nc.sync.dma_start(out=bqs, in_=bq_pm[:, :])
                xt_dma(nc.sync, 0, 0)
                xt_dma(nc.scalar, 0, 3)
                qk0[3]()                      # wk DMA (sync)
                nc.sync.dma_start(out=bks, in_=bk_pm[:, :])
                xt_dma(nc.sync, 1, 0)
                xt_dma(nc.scalar, 1, 3)
                nc.sync.dma_start(out=mask_sb, in_=maskc[:, :])
                nc.sync.dma_start(out=bos, in_=bo_bc[:, :])
                nc.sync.dma_start(out=ones64, in_=ones64c[:, :])
                qk0[1]()                      # q win0 matmuls
                wv_t = {}
                for ch in range(NCH):
                    w = wrow_p.tile([128, E], DT, tag="wrow")
                    nc.sync.dma_start(out=w, in_=_mm(wvT[ch * 128:(ch + 1) * 128, :]))
                    wv_t[ch] = w
                qk0[2]()                      # q win1
                for kc in range(NTC):
                    v65_r = v65_sb[:, kc, :].rearrange("p (h e) -> p h e", e=65)
                    nc.scalar.dma_start(out=v65_r[:, :, 64:65],
                                        in_=_mm(ones12[:, :]))
                qk0[4]()                      # k win0
                qk0[5]()                      # k win1
                for kc in range(4):
                    proj_v(kc)

                wo_t = {}

                def wo_dma(ch):
                    w = wrow_p.tile([128, E], DT, tag="wrow")
                    nc.sync.dma_start(out=w, in_=_mm(woT[ch * 128:(ch + 1) * 128, :]))
                    wo_t[ch] = w

                def out_tcn(tcn, use_stp=False):
                    if use_stp:
                        sto = stp.tile([128, 1024], F32, tag="st")
                        ps0 = sto[:, 0:512]
                        ps1 = sto[:, 512:768]
                    else:
                        ps0 = pp.tile([128, 512], F32, tag="pp")
                        ps1 = pp.tile([128, 256], F32, tag="pp")
                    for ch in range(NCH):
                        lhsT = ctxT_sb[:, ch, tcn * 128:(tcn + 1) * 128]
                        nc.tensor.matmul(ps0, lhsT, wo_t[ch][:, 0:512],
                                         start=(ch == 0), stop=(ch == NCH - 1))
                        nc.tensor.matmul(ps1, lhsT, wo_t[ch][:, 512:768],
                                         start=(ch == 0), stop=(ch == NCH - 1))
                    ot = out_p.tile([128, E], F32, tag="outp")
                    nc.vector.tensor_tensor(out=ot[:, 0:512], in0=ps0,
                                            in1=bos[:, 0:512], op=mybir.AluOpType.add)
                    nc.vector.tensor_tensor(out=ot[:, 512:768], in0=ps1,
                                            in1=bos[:, 512:768], op=mybir.AluOpType.add)
                    eng = nc.scalar if tcn % 2 else nc.sync
                    eng.dma_start(out=out[tcn * 128:(tcn + 1) * 128, :], in_=ot)

                units_q.extend(lambda kc=kc: proj_v(kc) for kc in range(4, NTC))
                units_q.extend(make_qk_units(1))
                attn_pair(0)
                for m in range(1, NCH):
                    if m < NCH - 1:
                        units_q.extend(make_qk_units(m + 1))
                    else:
                        # prefetch Wo, then (after 3 pad slots so the last
                        # pair's win0 normalize lands) t-chunks 0..3 of the
                        # output projection ride along in win1.
                        units_q.extend(lambda ch=ch: wo_dma(ch)
                                       for ch in range(NCH))
                        units_q.extend([lambda: None] * 3)
                        units_q.extend(lambda tcn=tcn: out_tcn(tcn)
                                       for tcn in range(4))
                    attn_pair(m)
                flush_all()

                # ---- output projection tail ----
                for tcn in range(4, NTC):
                    out_tcn(tcn, use_stp=(tcn % 2 == 1))

            for _rep in range(repeat):
                body()

    _split_excess_waits(nc)
    return nc


_NC_CACHE = None


def _make_in_maps(x, Wq, bq, Wk, bk, Wv, bv, Wo, bo):
    wq_r = np.ascontiguousarray(
        Wq.T.astype(np.float32).reshape(NCH, 128, NCH, 128)
        .transpose(2, 1, 0, 3)).astype(NP_DT)
    wk_r = np.ascontiguousarray(
        Wk.T.astype(np.float32).reshape(NCH, 128, NCH, 128)
        .transpose(2, 1, 0, 3)).astype(NP_DT)
    wvT = np.ascontiguousarray(Wv.T.astype(np.float32)).astype(NP_DT)
    woT = np.ascontiguousarray(Wo.T.astype(np.float32)).astype(NP_DT)
    bq_pm = np.ascontiguousarray(bq.reshape(NCH, 128).T.astype(np.float32))
    bk_pm = np.ascontiguousarray(bk.reshape(NCH, 128).T.astype(np.float32))
    bo_eff = (bo.astype(np.float64)
              + bv.astype(np.float64) @ Wo.T.astype(np.float64)).astype(np.float32)
    bo_bc = np.ascontiguousarray(np.tile(bo_eff[None, :], (128, 1)))
    maps = []
    for b in range(B):
        xTb = np.ascontiguousarray(x[b].T.astype(np.float32)).astype(NP_DT)
        maps.append({"xT": xTb, "wq_r": wq_r, "wk_r": wk_r, "wvT": wvT,
                     "woT": woT, "bq_pm": bq_pm, "bk_pm": bk_pm, "bo_bc": bo_bc})
    return maps


def kernel(x, Wq, bq, Wk, bk, Wv, bv, Wo, bo):
    global _NC_CACHE
    from concourse.bass_utils import run_bass_kernel_spmd
    if _NC_CACHE is None:
        _NC_CACHE = build_nc()
    in_maps = _make_in_maps(x, Wq, bq, Wk, bk, Wv, bv, Wo, bo)
    res = run_bass_kernel_spmd(_NC_CACHE, in_maps, core_ids=list(range(B)))
    return np.stack([res.results[i]["out"] for i in range(B)], axis=0)
